# revision 1
# baseline (speedup 1.0000x reference)
"""Trainium2 Bass kernel for nn_Attention_16707422781936.

Data-parallel over batch: B=8 -> one batch element per NeuronCore (8 cores).
Per core: qkv 1x1-conv GEMM, 8-head softmax attention over N=1600 tokens,
proj GEMM, depthwise 3x3 positional-encoding conv, summed output.
"""
import sys

sys.path.insert(0, "/opt/trn_rl_repo")

import ml_dtypes
import numpy as np

import concourse.bass as bass
import concourse.mybir as mybir
import concourse.tile as tile
from concourse import bacc
from concourse.bass_utils import run_bass_kernel_spmd

F32 = mybir.dt.float32
F32R = mybir.dt.float32r
BF16 = mybir.dt.bfloat16
ALU = mybir.AluOpType
EXP = mybir.ActivationFunctionType.Exp

C = 512          # channels
N = 1600         # tokens (40*40)
H = W = 40
NH = 8           # heads
KD = 32          # key dim
HD = 64          # head dim (v)
SCALE = KD ** -0.5

# n blocks (psum-bank sized) and m tiles (partition sized)
NB = [(0, 512), (512, 512), (1024, 512), (1536, 64)]
MT = [(i * 128, min(128, N - i * 128)) for i in range(13)]

_CACHE = {}


def build():
    nc = bacc.Bacc("TRN2", target_bir_lowering=False, debug=False,
                   enable_asserts=False)

    x_d = nc.dram_tensor("x", [C, N], BF16, kind="ExternalInput").ap()
    wqkvt_d = nc.dram_tensor("wqkvt", [128, 4 * 1024], BF16, kind="ExternalInput").ap()
    wprojt_d = nc.dram_tensor("wprojt", [128, 4 * 512], BF16, kind="ExternalInput").ap()
    bqk_d = nc.dram_tensor("bqk", [128, 4], F32, kind="ExternalInput").ap()
    bv_d = nc.dram_tensor("bv", [128, 4], F32, kind="ExternalInput").ap()
    bproj_d = nc.dram_tensor("bproj", [128, 4], F32, kind="ExternalInput").ap()
    bpe_d = nc.dram_tensor("bpe", [128, 4], F32, kind="ExternalInput").ap()
    wpe_d = nc.dram_tensor("wpe", [128, 36], F32, kind="ExternalInput").ap()
    ones_d = nc.dram_tensor("ones8", [128, NH], BF16, kind="ExternalInput").ap()
    out_d = nc.dram_tensor("out", [C, N], F32, kind="ExternalOutput").ap()

    with tile.TileContext(nc) as tc:
        with tc.tile_pool(name="persist", bufs=1) as per:
            qk_sb = per.tile([128, 4, N], BF16, tag="qk")      # q(h0-3),q(h4-7),k(h0-3),k(h4-7)
            v_sb = per.tile([128, 4, N], BF16, tag="v")         # v, channel-major (for dwconv)
            vt_sb = per.tile([128, 13, NH, HD + 1], BF16, tag="vt")  # v^T + ones col
            attn_sb = per.tile([128, 4, N], BF16, tag="attn")  # attention out, channel-major
            wprojt_sb = per.tile([128, 4, 512], BF16, tag="wprojt")
            bqk_sb = per.tile([128, 4], F32, tag="bqk")
            bv_sb = per.tile([128, 4], F32, tag="bv")
            bproj_sb = per.tile([128, 4], F32, tag="bproj")
            bpe_sb = per.tile([128, 4], F32, tag="bpe")
            wpe_sb = per.tile([128, 36], F32, tag="wpe")

            nc.sync.dma_start(wprojt_sb[:], wprojt_d.rearrange("p (t o) -> p t o", t=4))
            nc.sync.dma_start(bqk_sb[:], bqk_d)
            nc.sync.dma_start(bv_sb[:], bv_d)
            nc.sync.dma_start(bproj_sb[:], bproj_d)
            nc.sync.dma_start(bpe_sb[:], bpe_d)
            nc.sync.dma_start(wpe_sb[:], wpe_d)
            for mi in range(13):
                nc.sync.dma_start(vt_sb[:, mi, :, HD:HD + 1],
                                  ones_d.rearrange("p (h o) -> p h o", o=1))

            # ---------------- qkv GEMM phase (prefix only) ----------------
            # Only what attention pair 0 needs up front: q/k for heads 0-3
            # (mt 0 and 2) and the full v^T. The rest (q/k heads 4-7, v) is
            # dripped into the attention pipeline's PE slack below.
            x_sb = per.tile([128, 4, N], BF16, tag="x")
            wqkvt_sb = per.tile([128, 4, 1024], BF16, tag="wqkvt")
            x_dr = x_d.rearrange("(t p) n -> p t n", p=128)
            w_dr = wqkvt_d.rearrange("p (t o) -> p t o", t=4)
            for kt in range(4):
                nc.sync.dma_start(wqkvt_sb[:, kt], w_dr[:, kt])
            for (n0, nw) in NB:
                for kt in range(4):
                    nc.sync.dma_start(x_sb[:, kt, n0:n0 + nw],
                                      x_dr[:, kt, n0:n0 + nw])

            def qk_group(pool, tag, mt, n0, nw):
                qp = pool.tile([128, 512], F32, tag=tag, name="qg")
                for kt in range(4):
                    nc.tensor.matmul(
                        qp[0:128, 0:nw],
                        wqkvt_sb[:, kt, mt * 128:(mt + 1) * 128],
                        x_sb[:, kt, n0:n0 + nw],
                        start=(kt == 0), stop=(kt == 3))
                nc.vector.tensor_scalar(
                    out=qk_sb[:, mt, n0:n0 + nw], in0=qp[0:128, 0:nw],
                    scalar1=bqk_sb[:, mt:mt + 1], scalar2=None, op0=ALU.add)

            def v_group(pool, tag, ct, n0, nw):
                qp = pool.tile([128, 512], F32, tag=tag, name="vg")
                for kt in range(4):
                    nc.tensor.matmul(
                        qp[0:128, 0:nw],
                        wqkvt_sb[:, kt, 512 + ct * 128:512 + (ct + 1) * 128],
                        x_sb[:, kt, n0:n0 + nw],
                        start=(kt == 0), stop=(kt == 3))
                nc.vector.tensor_scalar(
                    out=v_sb[:, ct, n0:n0 + nw], in0=qp[0:128, 0:nw],
                    scalar1=bv_sb[:, ct:ct + 1], scalar2=None, op0=ALU.add)

            with tc.tile_pool(name="ps_qkv", bufs=4, space="PSUM") as psq:
                for mt in (0, 2):
                    for (n0, nw) in NB:
                        qk_group(psq, "qp", mt, n0, nw)
                for mi, (m0, mw) in enumerate(MT):
                    vp = psq.tile([128, 512], F32, tag="vp")
                    for kt in range(4):
                        nc.tensor.matmul(
                            vp[0:mw, 0:512],
                            x_sb[:, kt, m0:m0 + mw],
                            wqkvt_sb[:, kt, 512:1024],
                            start=(kt == 0), stop=(kt == 3))
                    nc.vector.tensor_copy(
                        out=vt_sb[0:mw, mi, :, 0:HD],
                        in_=vp[0:mw, 0:512].rearrange("p (h d) -> p h d", h=NH))

            # ---------------- depthwise 3x3 conv (VectorE) ----------------
            with tc.tile_pool(name="ph2", bufs=1) as ph2:
                pe_sb = ph2.tile([128, 4, H, W], F32, tag="pe")
                v4 = v_sb[:].rearrange("p t (h w) -> p t h w", h=H)

                # dwconv ops, generated lazily and dripped into attention
                def dwconv_ops():
                    for ct in range(4):
                        def center(ct=ct):
                            nc.vector.tensor_scalar(
                                out=pe_sb[:, ct], in0=v4[:, ct],
                                scalar1=wpe_sb[:, ct * 9 + 4:ct * 9 + 5],
                                scalar2=bpe_sb[:, ct:ct + 1],
                                op0=ALU.mult, op1=ALU.add)
                        yield center
                        for t in range(9):
                            dy, dx = t // 3 - 1, t % 3 - 1
                            if dy == 0 and dx == 0:
                                continue

                            def tap(ct=ct, t=t, dy=dy, dx=dx):
                                ys, ye = max(0, -dy), H - max(0, dy)
                                xs, xe = max(0, -dx), W - max(0, dx)
                                acc = pe_sb[:, ct, ys:ye, xs:xe]
                                nc.vector.scalar_tensor_tensor(
                                    out=acc,
                                    in0=v4[:, ct, ys + dy:ye + dy, xs + dx:xe + dx],
                                    scalar=wpe_sb[:, ct * 9 + t:ct * 9 + t + 1],
                                    in1=acc, op0=ALU.mult, op1=ALU.add)
                            yield tap

                # ---------------- attention + proj (flat pipeline) ----------
                pe3 = pe_sb[:].rearrange("p t h w -> p t (h w)")
                out_dr = out_d.rearrange("(t p) n -> p t n", p=128)
                with tc.tile_pool(name="ps_s", bufs=2, space="PSUM") as pss, \
                     tc.tile_pool(name="ps_av", bufs=4, space="PSUM") as psav, \
                     tc.tile_pool(name="expp", bufs=4) as expp, \
                     tc.tile_pool(name="nrm", bufs=4) as nrm, \
                     tc.tile_pool(name="outp", bufs=3) as outp:

                    def proj_ops(nbi):
                        n0, nw = NB[nbi]
                        for ot in range(4):
                            pp = [None]
                            for kt in range(4):
                                def mm(ot=ot, kt=kt, pp=pp):
                                    if kt == 0:
                                        pp[0] = psav.tile([128, 512], F32, tag="av", name="pp")
                                    nc.tensor.matmul(
                                        pp[0][0:128, 0:nw],
                                        wprojt_sb[:, kt, ot * 128:(ot + 1) * 128],
                                        attn_sb[:, kt, n0:n0 + nw],
                                        start=(kt == 0), stop=(kt == 3))
                                yield mm

                            def evac(ot=ot, pp=pp):
                                ob = outp.tile([128, 512], F32, tag="ob")
                                nc.vector.scalar_tensor_tensor(
                                    out=ob[0:128, 0:nw], in0=pp[0][0:128, 0:nw],
                                    scalar=bproj_sb[:, ot:ot + 1],
                                    in1=pe3[:, ot, n0:n0 + nw],
                                    op0=ALU.add, op1=ALU.add)
                                nc.sync.dma_start(out_dr[:, ot, n0:n0 + nw],
                                                  ob[0:128, 0:nw])
                            yield evac

                    def normalize(p, n0, nw, avs):
                        for j in range(2):
                            drow = nrm.tile([1, 512], F32, tag="drow")
                            dsplit = nrm.tile([32, 16], F32, tag="dsplit")
                            rsplit = nrm.tile([32, 16], F32, tag="rsplit")
                            rc = nrm.tile([1, 512], F32, tag="rc")
                            rb = nrm.tile([HD, 512], F32, tag="rb")
                            nws = nw // 32
                            nc.vector.tensor_copy(drow[0:1, 0:nw],
                                                  avs[j][HD:HD + 1, 0:nw])
                            nc.sync.dma_start(dsplit[0:32, 0:nws], drow[0:1, 0:nw])
                            nc.vector.reciprocal(rsplit[0:32, 0:nws], dsplit[0:32, 0:nws])
                            nc.sync.dma_start(rc[0:1, 0:nw], rsplit[0:32, 0:nws])
                            nc.gpsimd.partition_broadcast(rb[0:HD, 0:nw], rc[0:1, 0:nw])
                            nc.vector.scalar_tensor_tensor(
                                out=attn_sb[j * 64:j * 64 + 64, p, n0:n0 + nw],
                                in0=avs[j][0:HD, 0:nw], scalar=1.0, in1=rb[0:HD, 0:nw],
                                op0=ALU.bypass, op1=ALU.mult)

                    import collections as _c
                    drip = _c.deque(dwconv_ops())   # PE-free DVE drips
                    pe_drip = _c.deque()            # PE drips (proj matmuls)
                    # remaining qkv work, dripped one group per step at top
                    # priority: q/k of heads 4-7 first (needed by pair 2 at
                    # step 26), then v (only dwconv consumes it, later)
                    qkv_drip = _c.deque()
                    for (n0d, nwd) in NB:
                        qkv_drip.append(lambda n0=n0d, nw=nwd: qk_group(psav, "av", 1, n0, nw))
                    for (n0d, nwd) in NB:
                        qkv_drip.append(lambda n0=n0d, nw=nwd: qk_group(psav, "av", 3, n0, nw))
                    for ctd in range(4):
                        for (n0d, nwd) in NB:
                            qkv_drip.append(lambda ct=ctd, n0=n0d, nw=nwd: v_group(psav, "av", ct, n0, nw))
                    SWEEPS = [(nbi, p) for nbi in range(4) for p in range(4)]
                    pend = None  # deferred AV step: dict of sweep-step state

                    def emit_av(st):
                        for j in range(2):
                            nc.tensor.matmul(
                                st["avs"][j][0:HD + 1, 0:st["nw"]],
                                vt_sb[0:st["mw"], st["mi"], 2 * st["p"] + j, :],
                                st["es"][0:st["mw"], j * 512:j * 512 + st["nw"]],
                                start=(st["mi"] == 0), stop=(st["mi"] == 12))

                    proj_delay = _c.deque()

                    def retire(st):
                        emit_av(st)
                        if st["mi"] == 12:          # sweep finished
                            normalize(st["p"], st["n0"], st["nw"], st["avs"])
                            # release the previous nb's proj drip now that its
                            # normalize chains have had a sweep to complete
                            while proj_delay:
                                pe_drip.append(proj_delay.popleft())
                            if st["p"] == 3:        # all pairs done at this nb
                                proj_delay.extend(proj_ops(st["nbi"]))

                    for (nbi, p) in SWEEPS:
                        n0, nw = NB[nbi]
                        tq = p // 2
                        pb = (p % 2) * 64
                        avs = (psav.tile([HD + 1, 512], F32, tag="av", name="av0"),
                               psav.tile([HD + 1, 512], F32, tag="av", name="av1"))
                        for mi, (m0, mw) in enumerate(MT):
                            sp = pss.tile([128, 1024], F32, tag="sp")
                            for j in range(2):
                                nc.tensor.matmul(
                                    sp[0:mw, j * 512:j * 512 + nw],
                                    qk_sb[pb + 32 * j:pb + 32 * j + 32, 2 + tq, m0:m0 + mw],
                                    qk_sb[pb + 32 * j:pb + 32 * j + 32, tq, n0:n0 + nw],
                                    start=True, stop=True,
                                    tile_position=(pb + 32 * j, 0))
                            es = expp.tile([128, 1024], BF16, tag="es")
                            if nw == 512:
                                nc.scalar.activation(es[0:mw, :], sp[0:mw, :], EXP, scale=SCALE)
                            else:
                                sp3 = sp[:].rearrange("p (j n) -> p j n", j=2)
                                es3 = es[:].rearrange("p (j n) -> p j n", j=2)
                                nc.scalar.activation(es3[0:mw, :, 0:nw], sp3[0:mw, :, 0:nw],
                                                     EXP, scale=SCALE)
                            if pend is not None:
                                retire(pend)
                            if qkv_drip:
                                qkv_drip.popleft()()
                            elif pe_drip:
                                pe_drip.popleft()()
                            elif drip:
                                drip.popleft()()
                            pend = dict(avs=avs, p=p, nw=nw, mi=mi, mw=mw,
                                        es=es, n0=n0, nbi=nbi)
                    retire(pend)
                    while proj_delay:
                        pe_drip.append(proj_delay.popleft())
                    while pe_drip:
                        pe_drip.popleft()()
                    while drip:
                        drip.popleft()()

    nc.compile()
    return nc


def _prep(Wqkv, bqkv, Wproj, bproj, Wpe, bpe):
    WqkvT = np.ascontiguousarray(Wqkv.T)            # [512, 1024]
    wqkvt_h = np.ascontiguousarray(
        WqkvT.reshape(4, 128, 1024).transpose(1, 0, 2).reshape(128, 4096)
    ).astype(ml_dtypes.bfloat16)
    WprojT = np.ascontiguousarray(Wproj.T)          # [512, 512]
    wprojt_h = np.ascontiguousarray(
        WprojT.reshape(4, 128, 512).transpose(1, 0, 2).reshape(128, 2048)
    ).astype(ml_dtypes.bfloat16)
    bqk_h = np.ascontiguousarray(bqkv[0:512].reshape(4, 128).T)
    bv_h = np.ascontiguousarray(bqkv[512:1024].reshape(4, 128).T)
    # attention out is produced WITHOUT the v bias; Wproj @ bv is a constant
    # per output channel, so fold it into the proj bias on the host
    bproj_eff = bproj + Wproj @ bqkv[512:1024]
    bproj_h = np.ascontiguousarray(bproj_eff.reshape(4, 128).T)
    bpe_h = np.ascontiguousarray(bpe.reshape(4, 128).T)
    wpe_h = np.ascontiguousarray(
        Wpe.reshape(512, 9).reshape(4, 128, 9).transpose(1, 0, 2).reshape(128, 36))
    return dict(wqkvt=wqkvt_h, wprojt=wprojt_h, bqk=bqk_h, bv=bv_h,
                bproj=bproj_h, bpe=bpe_h, wpe=wpe_h,
                ones8=np.ones((128, NH), dtype=ml_dtypes.bfloat16))


def kernel(x, Wqkv, bqkv, Wproj, bproj, Wpe, bpe, _trace=False, _trace_kwargs=None):
    x = np.asarray(x, dtype=np.float32)
    Wqkv = np.asarray(Wqkv, dtype=np.float32)
    bqkv = np.asarray(bqkv, dtype=np.float32)
    Wproj = np.asarray(Wproj, dtype=np.float32)
    bproj = np.asarray(bproj, dtype=np.float32)
    Wpe = np.asarray(Wpe, dtype=np.float32)
    bpe = np.asarray(bpe, dtype=np.float32)
    B = x.shape[0]
    if "nc" not in _CACHE:
        _CACHE["nc"] = build()
    nc = _CACHE["nc"]
    shared = _prep(Wqkv, bqkv, Wproj, bproj, Wpe, bpe)
    xb = np.ascontiguousarray(x.reshape(B, C, N)).astype(ml_dtypes.bfloat16)
    in_maps = [dict(shared, x=xb[b]) for b in range(B)]
    res = run_bass_kernel_spmd(nc, in_maps, core_ids=list(range(8)),
                               trace=_trace, **(_trace_kwargs or {}))
    out = np.stack([res.results[b]["out"] for b in range(B)])
    kernel.last_result = res
    return out.reshape(B, C, H, W).astype(np.float32)



# revision 13
# speedup vs baseline: 1.0264x; 1.0264x over previous
"""Trainium2 Bass kernel for nn_Attention_16707422781936 (v2).

Data-parallel over batch: B=8 -> one batch element per NeuronCore.
Per core: qkv 1x1-conv GEMM, 8-head softmax attention over N=1600 tokens,
proj GEMM, depthwise 3x3 positional-encoding conv, summed output.

v2: the Activation engine's exp over the 8x1600x1600 score matrix is the
irreducible bottleneck (~190us), so all GEMMs move to fp8 DoubleRow
(scores: KD split 16+16 into the DR slots; AV + v^T: k-tile pairs; qkv
q/k: k-tile pairs with a sparse head layout so evacuation stays
lane-aligned; proj: channel-tile pairs) and the non-attention work is
dripped into per-step PE/DVE slack of the attention sweeps so the Act
engine runs from ~5us to the end without gaps. The 64-wide column tail
is one 8-heads-at-once sweep to keep exp instructions big. v (for the
positional-encoding dwconv) stays bf16 end-to-end for accuracy.
"""
import sys

sys.path.insert(0, "/opt/trn_rl_repo")

import ml_dtypes
import numpy as np

import concourse.bass as bass
import concourse.mybir as mybir
import concourse.tile as tile
from concourse import bacc
from concourse.bass_utils import run_bass_kernel_spmd

F32 = mybir.dt.float32
BF16 = mybir.dt.bfloat16
F8 = mybir.dt.float8e4
NPF8 = ml_dtypes.float8_e4m3
ALU = mybir.AluOpType
EXP = mybir.ActivationFunctionType.Exp
DR = mybir.MatmulPerfMode.DoubleRow

C = 512          # channels
N = 1600         # tokens (40*40)
H = W = 40
NH = 8           # heads
KD = 32          # key dim
HD = 64          # head dim (v)
SCALE = KD ** -0.5
NP = 1664        # key-padded length (13*128); pad keys have k=0

NBF = [(0, 512), (512, 512), (1024, 512)]
NB4 = NBF + [(1536, 64)]
TAIL0, TAILW = 1536, 64
MT = [(i * 128, 128) for i in range(13)]   # key tiles; mt12 reads zero-pad keys

_CACHE = {}


def build():
    nc = bacc.Bacc("TRN2", target_bir_lowering=False, debug=False,
                   enable_asserts=False)

    x_d = nc.dram_tensor("x", [C, N], BF16, kind="ExternalInput").ap()
    x8_d = nc.dram_tensor("x8", [C, N], F8, kind="ExternalInput").ap()
    wqk_d = nc.dram_tensor("wqk", [128, 8 * 2 * 2 * 128], F8, kind="ExternalInput").ap()
    wvt_d = nc.dram_tensor("wvt", [128, 4 * 512], BF16, kind="ExternalInput").ap()
    wvt8_d = nc.dram_tensor("wvt8", [128, 4 * 512], F8, kind="ExternalInput").ap()
    wproj_d = nc.dram_tensor("wproj", [128, 4 * 512], F8, kind="ExternalInput").ap()
    bqk_d = nc.dram_tensor("bqk", [128, 8], F32, kind="ExternalInput").ap()
    bv_d = nc.dram_tensor("bv", [128, 4], F32, kind="ExternalInput").ap()
    bpe_d = nc.dram_tensor("bpe", [128, 4], F32, kind="ExternalInput").ap()
    wpe_d = nc.dram_tensor("wpe", [128, 36], F32, kind="ExternalInput").ap()
    out_d = nc.dram_tensor("out", [C, N], F32, kind="ExternalOutput").ap()

    x_dr = x_d.rearrange("(t p) n -> p t n", p=128)
    x8_dr = x8_d.rearrange("(t p) n -> p t n", p=128)
    out_dr = out_d.rearrange("(t p) n -> p t n", p=128)

    with tile.TileContext(nc) as tc:
        with tc.tile_pool(name="per", bufs=1) as per, \
             tc.tile_pool(name="pss", bufs=2, space="PSUM") as pss, \
             tc.tile_pool(name="psav", bufs=4, space="PSUM") as psav, \
             tc.tile_pool(name="expp", bufs=8) as expp, \
             tc.tile_pool(name="nrm", bufs=4) as nrm, \
             tc.tile_pool(name="outp", bufs=3) as outp:

            x_sb = per.tile([128, 4, N], BF16, tag="x")
            x8_sb = per.tile([128, 4, N], F8, tag="x8")
            wqk_sb = per.tile([128, 8, 2, 2, 128], F8, tag="wqk")
            wvt_sb = per.tile([128, 4, 512], BF16, tag="wvt")
            wvt8_sb = per.tile([128, 2, 2, 512], F8, tag="wvt8")
            wproj_sb = per.tile([128, 4, 512], F8, tag="wproj")
            qF = per.tile([128, 2, 2, N], F8, tag="qF")     # [part, quad, slot, n]
            kF = per.tile([128, 2, 2, NP], F8, tag="kF")    # [part, quad, slot, m]
            v_sb = per.tile([128, 4, N], BF16, tag="v")
            vt_sb = per.tile([128, 13, NH, 96], F8, tag="vt")
            attn_sb = per.tile([128, 4, N], F8, tag="attn")
            pe_sb = per.tile([128, 4, H, W], F32, tag="pe")
            bqk_sb = per.tile([128, 8], F32, tag="bqk")
            bv_sb = per.tile([128, 4], F32, tag="bv")
            bpe_sb = per.tile([128, 4], F32, tag="bpe")
            wpe_sb = per.tile([128, 36], F32, tag="wpe")

            # input DMA: x8 nb0 + q/k weights first (they gate the prefix)
            nc.sync.dma_start(x8_sb[:, :, 0:512], x8_dr[:, :, 0:512])
            nc.sync.dma_start(
                wqk_sb[:], wqk_d.rearrange("p (g a s m) -> p g a s m", g=8, a=2, s=2))
            nc.sync.dma_start(bqk_sb[:], bqk_d)
            nc.sync.dma_start(
                wvt8_sb[:], wvt8_d.rearrange("p (a s o) -> p a s o", a=2, s=2))
            nc.sync.dma_start(x_sb[:, :, 0:512], x_dr[:, :, 0:512])
            for n0 in (512, 1024):
                nc.sync.dma_start(x8_sb[:, :, n0:n0 + 512], x8_dr[:, :, n0:n0 + 512])
                nc.sync.dma_start(x_sb[:, :, n0:n0 + 512], x_dr[:, :, n0:n0 + 512])
            nc.sync.dma_start(x8_sb[:, :, 1536:1600], x8_dr[:, :, 1536:1600])
            nc.sync.dma_start(x_sb[:, :, 1536:1600], x_dr[:, :, 1536:1600])
            nc.sync.dma_start(wvt_sb[:], wvt_d.rearrange("p (t o) -> p t o", t=4))
            nc.sync.dma_start(wproj_sb[:], wproj_d.rearrange("p (t o) -> p t o", t=4))
            nc.sync.dma_start(bv_sb[:], bv_d)
            nc.sync.dma_start(bpe_sb[:], bpe_d)
            nc.sync.dma_start(wpe_sb[:], wpe_d)

            # denominator ones column; zero pads (pad keys: score 0 -> es 1,
            # but their vt rows are zero so they contribute nothing)
            nc.vector.memset(vt_sb[:, :, :, HD:96], 0.0)
            nc.vector.memset(vt_sb[:, :, :, HD:HD + 1], 1.0)
            nc.vector.memset(vt_sb[64:128, 12, :, :], 0.0)
            nc.vector.memset(kF[:, :, :, N:NP], 0.0)

            # ---------------- work units ----------------
            def qk_unit(qk, quad, hh, nbi):
                """One sparse q/k group GEMM (fp8 DR over kt pairs) + evac."""
                n0, nw = NB4[nbi]
                g = qk * 4 + quad * 2 + hh
                qp = psav.tile([128, 512], F32, tag="av", name="qkg")
                for pr in range(2):
                    nc.tensor.matmul(
                        qp[0:128, 0:nw],
                        wqk_sb[:, g, pr, :, :],
                        x8_sb[:, 2 * pr:2 * pr + 2, n0:n0 + nw],
                        start=(pr == 0), stop=(pr == 1), perf_mode=DR)
                dst = kF if qk else qF
                nc.vector.tensor_scalar(
                    out=dst[:, quad, hh, n0:n0 + nw], in0=qp[0:128, 0:nw],
                    scalar1=bqk_sb[:, g:g + 1], scalar2=None, op0=ALU.add)

            def v_unit(ct, nbi):
                """v channel-major GEMM (bf16, for the dwconv), 2 drip halves."""
                n0, nw = NB4[nbi]
                st = [None]

                def half_a():
                    st[0] = psav.tile([128, 512], F32, tag="av", name="vg")
                    for kt in (0, 1):
                        nc.tensor.matmul(
                            st[0][0:128, 0:nw],
                            wvt_sb[:, kt, ct * 128:(ct + 1) * 128],
                            x_sb[:, kt, n0:n0 + nw],
                            start=(kt == 0), stop=False)

                def half_b():
                    for kt in (2, 3):
                        nc.tensor.matmul(
                            st[0][0:128, 0:nw],
                            wvt_sb[:, kt, ct * 128:(ct + 1) * 128],
                            x_sb[:, kt, n0:n0 + nw],
                            start=False, stop=(kt == 3))
                    nc.vector.tensor_scalar(
                        out=v_sb[:, ct, n0:n0 + nw], in0=st[0][0:128, 0:nw],
                        scalar1=bv_sb[:, ct:ct + 1], scalar2=None, op0=ALU.add)
                return half_a, half_b

            def vt_unit(mi):
                """v^T tile GEMM (fp8 DR over kt pairs) + fp8 evac."""
                m0, mw = mi * 128, min(128, N - mi * 128)
                vp = psav.tile([128, 512], F32, tag="av", name="vtg")
                for pr in range(2):
                    nc.tensor.matmul(
                        vp[0:mw, 0:512],
                        x8_sb[:, 2 * pr:2 * pr + 2, m0:m0 + mw],
                        wvt8_sb[:, pr, :, :],
                        start=(pr == 0), stop=(pr == 1), perf_mode=DR)
                nc.vector.tensor_copy(
                    out=vt_sb[0:mw, mi, :, 0:HD],
                    in_=vp[0:mw, 0:512].rearrange("p (h d) -> p h d", h=NH))

            v4 = v_sb[:].rearrange("p t (h w) -> p t h w", h=H)

            def dwconv_ops():
                for ct in range(4):
                    def center(ct=ct):
                        nc.vector.tensor_scalar(
                            out=pe_sb[:, ct], in0=v4[:, ct],
                            scalar1=wpe_sb[:, ct * 9 + 4:ct * 9 + 5],
                            scalar2=bpe_sb[:, ct:ct + 1],
                            op0=ALU.mult, op1=ALU.add)
                    yield center
                    for t in range(9):
                        dy, dx = t // 3 - 1, t % 3 - 1
                        if dy == 0 and dx == 0:
                            continue

                        def tap(ct=ct, t=t, dy=dy, dx=dx):
                            ys, ye = max(0, -dy), H - max(0, dy)
                            xs, xe = max(0, -dx), W - max(0, dx)
                            acc = pe_sb[:, ct, ys:ye, xs:xe]
                            nc.vector.scalar_tensor_tensor(
                                out=acc,
                                in0=v4[:, ct, ys + dy:ye + dy, xs + dx:xe + dx],
                                scalar=wpe_sb[:, ct * 9 + t:ct * 9 + t + 1],
                                in1=acc, op0=ALU.mult, op1=ALU.add)
                        yield tap

            pe3 = pe_sb[:].rearrange("p t h w -> p t (h w)")

            def proj_ops(n0, nw):
                for ot in range(4):
                    pp = [None]
                    for pr in range(2):
                        def mm(ot=ot, pr=pr, pp=pp):
                            if pr == 0:
                                pp[0] = psav.tile([128, 512], F32, tag="av", name="pp")
                            nc.tensor.matmul(
                                pp[0][0:128, 0:nw],
                                wproj_sb[:, 2 * pr:2 * pr + 2, ot * 128:(ot + 1) * 128],
                                attn_sb[:, 2 * pr:2 * pr + 2, n0:n0 + nw],
                                start=(pr == 0), stop=(pr == 1), perf_mode=DR)
                        yield mm

                    def evac(ot=ot, pp=pp):
                        ob = outp.tile([128, 512], F32, tag="ob")
                        nc.vector.scalar_tensor_tensor(
                            out=ob[0:128, 0:nw], in0=pp[0][0:128, 0:nw],
                            scalar=1.0 / 512.0,
                            in1=pe3[:, ot, n0:n0 + nw],
                            op0=ALU.mult, op1=ALU.add)
                        nc.sync.dma_start(out_dr[:, ot, n0:n0 + nw],
                                          ob[0:128, 0:nw])
                    yield evac

            def normalize(p, j, n0, nw, av):
                drow = nrm.tile([1, 512], F32, tag="drow")
                dsplit = nrm.tile([32, 16], F32, tag="dsplit")
                rsplit = nrm.tile([32, 16], F32, tag="rsplit")
                rc = nrm.tile([1, 512], F32, tag="rc")
                rb = nrm.tile([HD, 512], F32, tag="rb")
                nws = nw // 32
                nc.vector.tensor_copy(drow[0:1, 0:nw], av[HD:HD + 1, 0:nw])
                nc.sync.dma_start(dsplit[0:32, 0:nws], drow[0:1, 0:nw])
                nc.vector.reciprocal(rsplit[0:32, 0:nws], dsplit[0:32, 0:nws])
                nc.sync.dma_start(rc[0:1, 0:nw], rsplit[0:32, 0:nws])
                nc.gpsimd.partition_broadcast(rb[0:HD, 0:nw], rc[0:1, 0:nw])
                nc.vector.scalar_tensor_tensor(
                    out=attn_sb[j * 64:j * 64 + 64, p, n0:n0 + nw],
                    in0=av[0:HD, 0:nw], scalar=2.0, in1=rb[0:HD, 0:nw],
                    op0=ALU.mult, op1=ALU.mult)

            # ---------------- drip schedule ----------------
            import collections as _c
            pe_work = _c.deque()
            dve_work = _c.deque(dwconv_ops())
            proj_drip = _c.deque()

            for nbi in (1, 2, 3):           # k quad A, m 512.. (scores mt4+)
                pe_work.append(lambda n=nbi: qk_unit(1, 0, 0, n))
                pe_work.append(lambda n=nbi: qk_unit(1, 0, 1, n))
            for mi in (4, 5, 6, 7, 8, 9, 10, 11, 12):     # v^T (AV pair 2+)
                pe_work.append(lambda m=mi: vt_unit(m))
            pe_work.append(lambda: qk_unit(0, 1, 0, 0))   # q quad B nb0
            pe_work.append(lambda: qk_unit(0, 1, 1, 0))
            for nbi in (0, 1, 2, 3):                      # k quad B all m
                pe_work.append(lambda n=nbi: qk_unit(1, 1, 0, n))
                pe_work.append(lambda n=nbi: qk_unit(1, 1, 1, n))
            for nbi in (1, 2):                            # q for later blocks
                for quad in (0, 1):
                    pe_work.append(lambda q=quad, n=nbi: qk_unit(0, q, 0, n))
                    pe_work.append(lambda q=quad, n=nbi: qk_unit(0, q, 1, n))
            for ct in range(4):                           # v (dwconv source)
                for nbi in range(4):
                    ha, hb = v_unit(ct, nbi)
                    pe_work.append(ha)
                    pe_work.append(hb)
            for quad in (0, 1):                           # q for tail block
                pe_work.append(lambda q=quad: qk_unit(0, q, 0, 3))
                pe_work.append(lambda q=quad: qk_unit(0, q, 1, 3))

            step = [0]

            def drip():
                step[0] += 1
                if pe_work:
                    pe_work.popleft()()
                elif proj_drip:
                    proj_drip.popleft()()
                elif dve_work:
                    dve_work.popleft()()
                # dwconv paced on DVE once its v inputs are safely dripped
                if step[0] >= 45 and dve_work and (pe_work or proj_drip):
                    dve_work.popleft()()

            # ---------------- prefix ----------------
            qk_unit(1, 0, 0, 0)
            qk_unit(1, 0, 1, 0)
            qk_unit(0, 0, 0, 0)
            qk_unit(0, 0, 1, 0)
            for mi in (0, 1, 2, 3):
                vt_unit(mi)

            # ---------------- attention sweeps ----------------
            for nbi, (n0, nw) in enumerate(NB4):
                for p in range(4):
                    if nbi > 0 and p == 1:   # release prev block's projection
                        proj_drip.extend(proj_ops(*NBF[nbi - 1]))
                    quad, h0, h1 = p // 2, (2 * p) % 4, (2 * p + 1) % 4
                    avs = (psav.tile([96, 512], F32, tag="av", name="av0"),
                           psav.tile([96, 512], F32, tag="av", name="av1"))
                    es = None
                    for mi, (m0, mw) in enumerate(MT):
                        sp = pss.tile([128, 2, 512], F32, tag="sp")
                        for j, hj in ((0, h0), (1, h1)):
                            nc.tensor.matmul(
                                sp[0:mw, j, 0:nw],
                                kF[32 * hj:32 * hj + 16, quad, :, m0:m0 + mw],
                                qF[32 * hj:32 * hj + 16, quad, :, n0:n0 + nw],
                                start=True, stop=True, perf_mode=DR,
                                tile_position=(32 * hj, 0))
                        if mi % 2 == 0:
                            es = expp.tile([128, 2, 2, 512], F8, tag="es")
                        nc.scalar.activation(es[0:mw, mi % 2, :, 0:nw],
                                             sp[0:mw, :, 0:nw], EXP, scale=SCALE / 256.0)
                        if mi % 2 == 1:
                            pi = mi // 2
                            for j in range(2):
                                nc.tensor.matmul(
                                    avs[j][0:96, 0:nw],
                                    vt_sb[0:128, 2 * pi:2 * pi + 2, 2 * p + j, :],
                                    es[0:128, 0:2, j, 0:nw],
                                    start=(pi == 0), stop=False, perf_mode=DR)
                        elif mi == 12:
                            for j in range(2):
                                nc.tensor.matmul(
                                    avs[j][0:96, 0:nw],
                                    vt_sb[0:128, 12, 2 * p + j, :],
                                    es[0:128, 0, j, 0:nw],
                                    start=False, stop=True)
                        drip()
                        if nbi == 0 and p == 0:
                            drip()
                    for j in range(2):
                        normalize(p, j, n0, nw, avs[j])

            while pe_work or proj_drip or dve_work:
                drip()
            for op in proj_ops(TAIL0, TAILW):
                op()

    nc.compile()
    return nc


def _prep(Wqkv, bqkv, Wproj, bproj, Wpe, bpe):
    WqkvT = np.ascontiguousarray(Wqkv.T)            # [512 c, 1024 out]
    # q/k sparse groups: g = qk*4 + quad*2 + hh. Device lhsT element
    # [p, g, pair, slot, col] = WqkvT[(2*pair+slot)*128 + p, ch(col)] where
    # col = 32*h' + j (j<16) maps to channel qk*256 + (4*quad+h')*32 + 16*hh + j
    # and cols 32*h'+16..31 are zero (junk lanes).
    wqk_dev = np.zeros((128, 8, 2, 2, 128), dtype=np.float32)
    bqk = np.zeros((128, 8), dtype=np.float32)
    j16 = np.arange(16)
    for g in range(8):
        qk, quad, hh = g // 4, (g % 4) // 2, g % 2
        for hp in range(4):
            ch = qk * 256 + (4 * quad + hp) * 32 + 16 * hh + j16
            cols = 32 * hp + j16
            for pair in range(2):
                for slot in range(2):
                    kt = 2 * pair + slot
                    wqk_dev[:, g, pair, slot, cols] = 16.0 * WqkvT[kt * 128:(kt + 1) * 128, ch]
            bqk[cols, g] = 16.0 * bqkv[ch]
    wqk_f8 = np.ascontiguousarray(wqk_dev.reshape(128, 8 * 2 * 2 * 128)).astype(NPF8)

    wvt_np = WqkvT[:, 512:1024]                     # [512 c, 512 vch]
    wvt_h = np.ascontiguousarray(
        wvt_np.reshape(4, 128, 512).transpose(1, 0, 2).reshape(128, 2048)
    ).astype(ml_dtypes.bfloat16)
    # fp8 copy in [p, pair, slot, o] order: [p, (2*pair+slot chunk), o]
    wvt8_h = np.ascontiguousarray(
        16.0 * wvt_np.reshape(4, 128, 512).transpose(1, 0, 2).reshape(128, 2048)
    ).astype(NPF8)
    WprojT = np.ascontiguousarray(Wproj.T)          # [512, 512]
    wproj_h = np.ascontiguousarray(
        16.0 * WprojT.reshape(4, 128, 512).transpose(1, 0, 2).reshape(128, 2048)
    ).astype(NPF8)
    bv_h = np.ascontiguousarray(bqkv[512:1024].reshape(4, 128).T)
    bproj_eff = bproj + Wproj @ bqkv[512:1024]
    bpe_h = np.ascontiguousarray((bpe + bproj_eff).reshape(4, 128).T)
    wpe_h = np.ascontiguousarray(
        Wpe.reshape(512, 9).reshape(4, 128, 9).transpose(1, 0, 2).reshape(128, 36))
    return dict(wqk=wqk_f8, wvt=wvt_h, wvt8=wvt8_h, wproj=wproj_h,
                bqk=np.ascontiguousarray(bqk, dtype=np.float32),
                bv=bv_h.astype(np.float32),
                bpe=bpe_h.astype(np.float32), wpe=wpe_h.astype(np.float32))


def kernel(x, Wqkv, bqkv, Wproj, bproj, Wpe, bpe, _trace=False, _trace_kwargs=None):
    x = np.asarray(x, dtype=np.float32)
    Wqkv = np.asarray(Wqkv, dtype=np.float32)
    bqkv = np.asarray(bqkv, dtype=np.float32)
    Wproj = np.asarray(Wproj, dtype=np.float32)
    bproj = np.asarray(bproj, dtype=np.float32)
    Wpe = np.asarray(Wpe, dtype=np.float32)
    bpe = np.asarray(bpe, dtype=np.float32)
    B = x.shape[0]
    if "nc" not in _CACHE:
        _CACHE["nc"] = build()
    nc = _CACHE["nc"]
    shared = _prep(Wqkv, bqkv, Wproj, bproj, Wpe, bpe)
    xb = np.ascontiguousarray(x.reshape(B, C, N))
    in_maps = [dict(shared, x=xb[b].astype(ml_dtypes.bfloat16),
                    x8=xb[b].astype(NPF8)) for b in range(B)]
    res = run_bass_kernel_spmd(nc, in_maps, core_ids=list(range(8)),
                               trace=_trace, **(_trace_kwargs or {}))
    out = np.stack([np.asarray(res.results[b]["out"], dtype=np.float32)
                    for b in range(B)])
    kernel.last_result = res
    return out.reshape(B, C, H, W)
